# revision 9
# baseline (speedup 1.0000x reference)
"""Bass/Trainium2 kernel for the LSTM problem (nn_RNN_27685359190558).

Math (per reference):
  xW = x @ W + b                      [B, T, 4H]
  scan over T=28: z = xW_t + h @ U; i,f,g,o = split(z) (Keras order)
      i,f,o = sigmoid; g = relu
      c' = f*c + i*g;  h' = o * relu(c')
  out = softmax(h_final @ Wd + bd)    [B, 10]

Strategy: pure data parallelism over 8 cores (2048 batch each).
On-chip layout is fully transposed ("orientation A"): states hT/cT are
[H=128 partitions, batch free].  Per (timestep, 512-batch chunk) and per
gate q: psum[q] = Wt[:,q].T @ xT_t + Ur[:,q].T @ hT  (fp16 matmuls,
fp32 psum accumulate; K=29 / K=128).  fp16 streams 1 PE row/cycle vs
~3.7 for fp32r, and keeps rel err ~6e-4 (threshold 2e-2).  Gate order
in psum is [i, f, o, g] so one fused ACT sigmoid covers [128, 1536].
Since c0=0 and c' = f*c + i*relu(g) with f,i>0, c stays >= 0, so
relu(c)=c and h' = o*c is a plain tensor_tensor (all-fp16 fast path).
Bias b is folded in via a ones-row appended to x (host side).
Dense + softmax run at the end (one ACT table switch).
"""

import sys

sys.path.insert(0, "/opt/trn_rl_repo")

import numpy as np
from contextlib import ExitStack

import concourse.bass as bass
import concourse.bacc as bacc
import concourse.tile as tile
from concourse import mybir
from concourse.bass_utils import run_bass_kernel_spmd

B, T, F, H = 16384, 28, 28, 128
G = 4 * H  # 512
NCLS = 10
NCORES = 8
BC = B // NCORES  # 2048 batch per core
CH = 512  # batch chunk per matmul (one psum bank)
NCH = BC // CH  # 4
FP = F + 1  # 29: features + ones row (bias)

FP32 = mybir.dt.float32
FP16 = mybir.dt.float16  # DVE-side states (c, sigmoid outs, t1): better mantissa
BF16 = mybir.dt.bfloat16  # matmul operands: full-rate on the PE (fp16 is half-rate)

TRACE = False
TIME_REPS = 0  # >0: run cached-executable wall-clock timing after correctness run
LAST_RESULT = None


def _build_kernel(ctx, tc, xT, Wt, Ur, Wd, bd, ones1h, out, skip_bias):
    nc = tc.nc
    Sig = mybir.ActivationFunctionType.Sigmoid
    Exp = mybir.ActivationFunctionType.Exp
    mul_op = mybir.AluOpType.mult
    add_op = mybir.AluOpType.add
    max_op = mybir.AluOpType.max

    weights = ctx.enter_context(tc.tile_pool(name="weights", bufs=1))
    state = ctx.enter_context(tc.tile_pool(name="state", bufs=1))
    xpool = ctx.enter_context(tc.tile_pool(name="xpool", bufs=1))
    spool = ctx.enter_context(tc.tile_pool(name="spool", bufs=6))
    tpool = ctx.enter_context(tc.tile_pool(name="tpool", bufs=6))
    opool = ctx.enter_context(tc.tile_pool(name="opool", bufs=2))

    dma = nc.default_dma_engine

    wt_sb = weights.tile([H, G], FP16)
    ur_sb = weights.tile([H, G], FP16)
    wd_sb = weights.tile([H, NCLS], FP16)
    bd_sb = weights.tile([1, NCLS], FP16)
    ones_sb = weights.tile([1, H], FP16)
    # DMA order = queue order: wt first (gates t=0), then x0 per-chunk so the
    # first W-matmul starts after ~1/4 of x0 lands, THEN the U/dense weights
    # (not needed until t=1 / the tail) — trims the PE startup stall.
    dma.dma_start(out=wt_sb[:], in_=Wt[:])
    # x tiles are zero-padded from K=FP(29) to K=128: matmuls with K<128
    # stream at HALF the PE column rate (measured 629 vs 379 ns / 512 cols),
    # and the padded rows multiply by the zero rows of Wt.  4 rotating
    # buffers, padding memset once, DMA rewrites only rows 0..FP-1.
    xbufs = [xpool.tile([H, BC], FP16, name=f"xtbuf{j}") for j in range(4)]
    for j in range(4):
        nc.gpsimd.memset(xbufs[j][:], 0.0)
    xt0 = xbufs[0]
    for c in range(NCH):
        dma.dma_start(out=xt0[0:FP, c * CH : (c + 1) * CH], in_=xT[0][:, c * CH : (c + 1) * CH])
    dma.dma_start(out=ur_sb[:], in_=Ur[:])
    dma.dma_start(out=wd_sb[:], in_=Wd[:])
    dma.dma_start(out=bd_sb[:], in_=bd[:])
    dma.dma_start(out=ones_sb[:], in_=ones1h[:])

    hT = state.tile([H, BC], FP16)
    cT = state.tile([H, BC], FP16)

    def finish_chunk(c, s, t1):
        # Whole c-chain stays on DVE in program order (t1, f*c, c+=t1) so the
        # in-order DVE stream never blocks on a slow Pool hop mid-chain; only
        # the terminal h = c*o runs on Pool, off the DVE critical path (its
        # lone consumer is next-t's U-matmul, absorbed by chunk pipelining).
        c0, c1 = c * CH, (c + 1) * CH
        nc.vector.tensor_tensor(
            out=cT[:, c0:c1], in0=cT[:, c0:c1], in1=t1[:], op=add_op
        )
        nc.gpsimd.tensor_tensor(
            out=hT[:, c0:c1],
            in0=cT[:, c0:c1],
            in1=s[:, 2 * CH : 3 * CH],
            op=mul_op,
        )

    with (
        tc.tile_pool(name="ppool", bufs=2, space="PSUM") as ppool,
        tc.tile_pool(name="gpool", bufs=2, space="PSUM") as gpool,
    ):
        for t in range(T):
            if t == 0:
                xt = xt0
            else:
                xt = xbufs[t % 4]
                dma.dma_start(out=xt[0:FP, :], in_=xT[t])
            pending = None
            for c in range(NCH):
                c0, c1 = c * CH, (c + 1) * CH
                pt = ppool.tile([H, 3 * CH], FP32)
                pg = gpool.tile([H, CH], FP32)
                # U-matmul FIRST so the psum accumulation group (and pool
                # slot) opens as late as possible — psum residency, not
                # engine busy, limits chunk-level parallelism.
                for q in range(4):
                    dst = pt[:, q * CH : (q + 1) * CH] if q < 3 else pg[:]
                    if t > 0:
                        nc.tensor.matmul(
                            dst,
                            ur_sb[:, q * H : (q + 1) * H],
                            hT[:, c0:c1],
                            start=True,
                            stop=False,
                        )
                    nc.tensor.matmul(
                        dst,
                        wt_sb[:, q * H : (q + 1) * H],
                        xt[:, c0:c1],
                        start=(t == 0),
                        stop=True,
                    )
                s = spool.tile([H, 3 * CH], FP16)
                nc.scalar.activation(out=s[:], in_=pt[:], func=Sig)
                if t == 0:
                    # c0 = 0  =>  c' = i * relu(g) = relu(i*g);  h = o*c
                    nc.vector.scalar_tensor_tensor(
                        out=cT[:, c0:c1],
                        in0=pg[:],
                        scalar=0.0,
                        in1=s[:, 0:CH],
                        op0=max_op,
                        op1=mul_op,
                    )
                    nc.vector.tensor_tensor(
                        out=hT[:, c0:c1],
                        in0=cT[:, c0:c1],
                        in1=s[:, 2 * CH : 3 * CH],
                        op=mul_op,
                    )
                else:
                    t1 = tpool.tile([H, CH], FP16)
                    nc.vector.scalar_tensor_tensor(
                        out=t1[:],
                        in0=pg[:],
                        scalar=0.0,
                        in1=s[:, 0:CH],
                        op0=max_op,
                        op1=mul_op,
                    )
                    # f*c on DVE (all-fp16 SBUF operands hit the 2x_1p mode)
                    nc.vector.tensor_tensor(
                        out=cT[:, c0:c1],
                        in0=s[:, CH : 2 * CH],
                        in1=cT[:, c0:c1],
                        op=mul_op,
                    )
                    if pending is not None:
                        finish_chunk(*pending)
                    pending = (c, s, t1)
            if pending is not None:
                finish_chunk(*pending)

        # dense + softmax, inside the psum pools' scope (reusing a gpool
        # slot) so no pool-close barrier separates it from the last steps.
        # All 16 batch-blocks' logits land in ONE [128, 160] psum tile
        # (block j at cols 10j..10j+10), so softmax is one wide exp, one
        # 3D-grouped reduce, one reciprocal, one broadcast multiply —
        # instead of 16 serialized per-block ACT/DVE chains.
        NB = BC // H  # 16
        pg = gpool.tile([H, CH], FP32)
        pw = pg[:, 0 : NB * NCLS]
        for j in range(NB):
            d0 = j * NCLS
            nc.tensor.matmul(
                pw[:, d0 : d0 + NCLS],
                hT[:, j * H : (j + 1) * H],
                wd_sb[:],
                start=True,
                stop=skip_bias,
            )
            if not skip_bias:
                # + bd via a rank-1 ones @ bd matmul (keeps bias off the DVE)
                nc.tensor.matmul(
                    pw[:, d0 : d0 + NCLS], ones_sb[:], bd_sb[:], start=False, stop=True
                )
        # logits are O(1) (sigmoid-gated h, small Wd) — skip max-subtract
        ex = opool.tile([H, NB * NCLS], FP32)
        nc.scalar.activation(out=ex[:], in_=pw[:], func=Exp)
        ex3 = ex[:].rearrange("p (g k) -> p g k", g=NB)
        sm = opool.tile([H, NB], FP32)
        nc.vector.tensor_reduce(
            out=sm[:], in_=ex3, axis=mybir.AxisListType.X, op=add_op
        )
        rc = opool.tile([H, NB], FP32)
        nc.vector.reciprocal(out=rc[:], in_=sm[:])
        pr = opool.tile([H, NB * NCLS], FP32)
        nc.vector.tensor_tensor(
            out=pr[:].rearrange("p (g k) -> p g k", g=NB),
            in0=ex3,
            in1=rc[:].unsqueeze(2).broadcast_to([H, NB, NCLS]),
            op=mul_op,
        )
        dma.dma_start(
            out=out[:].rearrange("(g p) k -> p g k", g=NB),
            in_=pr[:].rearrange("p (g k) -> p g k", g=NB),
        )


def _build_nc(skip_bias):
    nc = bacc.Bacc(None, target_bir_lowering=False, debug=False)
    xT = nc.declare_dram_parameter("xT", [T, FP, BC], FP16, isOutput=False)
    Wt = nc.declare_dram_parameter("Wt", [H, G], FP16, isOutput=False)
    Ur = nc.declare_dram_parameter("Ur", [H, G], FP16, isOutput=False)
    Wd = nc.declare_dram_parameter("Wd", [H, NCLS], FP16, isOutput=False)
    bd = nc.declare_dram_parameter("bd", [1, NCLS], FP16, isOutput=False)
    ones1h = nc.declare_dram_parameter("ones1h", [1, H], FP16, isOutput=False)
    out = nc.declare_dram_parameter("out", [BC, NCLS], FP32, isOutput=True)

    with tile.TileContext(nc) as tc, ExitStack() as ctx:
        _build_kernel(ctx, tc, xT, Wt, Ur, Wd, bd, ones1h, out, skip_bias)
    return nc


# psum/sigmoid gate order [i, f, o, g]; W/U columns are [i, f, g, o]
_GATE_PERM = np.concatenate(
    [np.arange(0, 2 * H), np.arange(3 * H, 4 * H), np.arange(2 * H, 3 * H)]
)


def _run_timed(nc, in_maps, n_cores, reps):
    """Cached-executable min-of-N wall timing (NTFF unavailable under axon).

    Mirrors bass2jax.run_bass_via_pjrt's multi-core path but jits WITHOUT
    donation (our kernel writes every output element, so zero-init buffers
    are not needed) and keeps all operands device-resident across reps.
    """
    import time as _time

    import jax
    from jax.experimental.shard_map import shard_map
    from jax.sharding import Mesh, NamedSharding, PartitionSpec

    from concourse import bass2jax

    bass2jax.install_neuronx_cc_hook()
    partition_name = nc.partition_id_tensor.name if nc.partition_id_tensor else None

    in_names, out_names, out_avals, zero_outs = [], [], [], []
    for alloc in nc.m.functions[0].allocations:
        if not isinstance(alloc, mybir.MemoryLocationSet):
            continue
        name = alloc.memorylocations[0].name
        if alloc.kind == "ExternalInput":
            if name != partition_name:
                in_names.append(name)
        elif alloc.kind == "ExternalOutput":
            out_names.append(name)
            shape = tuple(alloc.tensor_shape)
            dtype = mybir.dt.np(alloc.dtype)
            out_avals.append(jax.core.ShapedArray(shape, dtype))
            zero_outs.append(np.zeros(shape, dtype))
    n_params = len(in_names)
    in_names = in_names + out_names
    if partition_name is not None:
        in_names.append(partition_name)

    def _body(*args):
        operands = list(args)
        if partition_name is not None:
            operands.append(bass2jax.partition_id_tensor())
        return tuple(
            bass2jax._bass_exec_p.bind(
                *operands,
                out_avals=tuple(out_avals),
                in_names=tuple(in_names),
                out_names=tuple(out_names),
                lowering_input_output_aliases=(),
                sim_require_finite=True,
                sim_require_nnan=True,
                nc=nc,
            )
        )

    devices = jax.devices()[:n_cores]
    mesh = Mesh(np.asarray(devices), ("core",))
    nsh = NamedSharding(mesh, PartitionSpec("core"))
    in_specs = (PartitionSpec("core"),) * (n_params + len(out_names))
    out_specs = (PartitionSpec("core"),) * len(out_names)
    sharded = jax.jit(
        shard_map(
            _body, mesh=mesh, in_specs=in_specs, out_specs=out_specs, check_rep=False
        ),
        keep_unused=True,
    )
    per_core = [[np.asarray(m[name]) for name in in_names[:n_params]] for m in in_maps]
    concat_in = [
        np.concatenate([per_core[c][i] for c in range(n_cores)], axis=0)
        for i in range(n_params)
    ]
    concat_zeros = [
        np.zeros((n_cores * z.shape[0], *z.shape[1:]), z.dtype) for z in zero_outs
    ]
    args_dev = [jax.device_put(a, nsh) for a in concat_in + concat_zeros]
    out = jax.block_until_ready(sharded(*args_dev))  # compile + warmup
    times = []
    for _ in range(reps):
        t0 = _time.perf_counter_ns()
        o = jax.block_until_ready(sharded(*args_dev))
        times.append(_time.perf_counter_ns() - t0)
    results = [
        {
            name: np.asarray(out[i]).reshape(n_cores, *out_avals[i].shape)[c]
            for i, name in enumerate(out_names)
        }
        for c in range(n_cores)
    ]
    return results, min(times), sum(times) / len(times)


def kernel(x, W, U, b, Wd, bd):
    global LAST_RESULT
    x = np.ascontiguousarray(np.asarray(x, dtype=np.float32))
    W = np.asarray(W, dtype=np.float32)
    U = np.asarray(U, dtype=np.float32)
    b = np.asarray(b, dtype=np.float32)
    Wd = np.ascontiguousarray(np.asarray(Wd, dtype=np.float32))
    bd = np.asarray(bd, dtype=np.float32)

    bf16 = np.float16
    Wt_host = np.ascontiguousarray(
        np.vstack([W, b[None, :], np.zeros((H - FP, G), np.float32)])[
            :, _GATE_PERM
        ].astype(bf16)
    )
    Ur_host = np.ascontiguousarray(U[:, _GATE_PERM].astype(bf16))
    Wd_host = np.ascontiguousarray(Wd.astype(bf16))
    bd_host = np.ascontiguousarray(bd.reshape(1, NCLS).astype(bf16))

    xs = x.reshape(NCORES, BC, T, F)
    in_maps = []
    for ci in range(NCORES):
        xc = xs[ci].transpose(1, 2, 0).astype(bf16)  # [T, F, BC]
        xTc = np.concatenate(
            [xc, np.ones((T, 1, BC), dtype=bf16)], axis=1
        )  # [T, FP, BC]
        in_maps.append(
            {
                "xT": np.ascontiguousarray(xTc),
                "Wt": Wt_host,
                "Ur": Ur_host,
                "Wd": Wd_host,
                "bd": bd_host,
                "ones1h": np.ones((1, H), dtype=bf16),
            }
        )

    nc = _build_nc(skip_bias=not np.any(bd))
    nc.finalize()
    if TIME_REPS > 0:
        from concourse.bass_utils import BassKernelResults

        results, min_ns, mean_ns = _run_timed(nc, in_maps, NCORES, TIME_REPS)
        res = BassKernelResults(
            results=results,
            instructions_and_trace=None,
            profile_json=None,
            exec_time_ns=int(min_ns),
            mean_exec_time_ns=mean_ns,
        )
    else:
        res = run_bass_kernel_spmd(nc, in_maps, list(range(NCORES)), trace=TRACE)
    LAST_RESULT = res
    out = np.concatenate([res.results[i]["out"] for i in range(NCORES)], axis=0)
    return np.ascontiguousarray(out.astype(np.float32))
